# revision 1
# baseline (speedup 1.0000x reference)
"""GraphQLayer fused kernel for 8x trn2 NeuronCores.

Math reduction: the reference output is rank-1.
  fid = (x @ x.T)^2 ; adj = (fid >= 0.85), zero diag
  agg_scalar[i] = mean_d( (adj @ x)[i, :] ) = sum_j adj[i,j] * s[j] / 128,
  with s[j] = sum_d x[j, d].  out[i, h] = agg_scalar[i] * wsum[h] + b[h],
  wsum[h] = sum_d W[h, d].
So per core we need only:  G = x @ x.T (slab), mask = |G| >= g0
(g0 = minimal fp32 y with fl(y*y) >= 0.85, exact threshold equivalence),
then a masked weighted column-sum  agg = mask.T-reduce with weights s/128 —
done on the PE as small matmuls with the s-column as the stationary operand.
Diagonal correction and bias are folded into a host-precomputed [2048, 64]
additive term.  Sharding: row-shard the Gram over 8 cores (each core owns a
2048-column slab of x.T as the moving operand); x.T replicated.
"""

import sys
from contextlib import ExitStack

import numpy as np

sys.path.insert(0, "/opt/trn_rl_repo")

import concourse.bass as bass  # noqa: E402
import concourse.tile as tile  # noqa: E402
from concourse import bacc, mybir  # noqa: E402
from concourse.bass_utils import run_bass_kernel_spmd  # noqa: E402

N, D, H = 16384, 128, 64
NCORES = 8
MSLAB = N // NCORES          # 2048 output rows per core
MCHUNK = 512                 # m columns per pass (PSUM bank width)
NPASS = MSLAB // MCHUNK      # 4
NB = N // 128                # 128 n-blocks
THRESH = 0.85

f32 = mybir.dt.float32
f32r = mybir.dt.float32r
bf16 = mybir.dt.bfloat16
i32 = mybir.dt.int32
AOP = mybir.AluOpType
AFT = mybir.ActivationFunctionType


def _round_f32r(a: np.ndarray) -> np.ndarray:
    """Round fp32 to the fp32r-representable set (sum of two bf16 parts)."""
    import ml_dtypes
    hi = a.astype(ml_dtypes.bfloat16).astype(np.float32)
    lo = (a - hi).astype(ml_dtypes.bfloat16).astype(np.float32)
    return (hi + lo).astype(np.float32)


def _abs_threshold() -> float:
    """Minimal fp32 y such that fl(y*y) >= 0.85 (exact mask equivalence)."""
    y = np.float32(np.sqrt(np.float64(THRESH)))
    thr = np.float32(THRESH)
    while np.float32(y * y) >= thr:
        y = np.nextafter(y, np.float32(0.0))
    while np.float32(y * y) < thr:
        y = np.nextafter(y, np.float32(np.inf))
    return float(y)


GS = 2  # n-blocks per PSUM drain group


def _groups():
    gs = []
    nb = 0
    while nb < NB:
        gs.append(list(range(nb, min(nb + GS, NB))))
        nb += GS
    return gs


def _build_kernel(c_abs: float):
    nc = bacc.Bacc("TRN2", target_bir_lowering=False, debug=False,
                   num_devices=NCORES)
    xt_d = nc.dram_tensor("xt", [128, N], f32, kind="ExternalInput").ap()
    xts_d = nc.dram_tensor("xts", [128, MSLAB], f32, kind="ExternalInput").ap()
    scols_d = nc.dram_tensor("scols", [128, 2 * NB], bf16, kind="ExternalInput").ap()
    wsum_d = nc.dram_tensor("wsum", [128, H], f32, kind="ExternalInput").ap()
    addc_d = nc.dram_tensor("addc", [MSLAB, H], f32, kind="ExternalInput").ap()
    out_d = nc.dram_tensor("out", [MSLAB, H], f32, kind="ExternalOutput").ap()

    with tile.TileContext(nc) as tc:
        with ExitStack() as ctx:
            _emit(ctx, tc, out_d, xt_d, xts_d, scols_d, wsum_d, addc_d, c_abs)
    nc.compile()
    return nc


def _emit(ctx, tc, out_d, xt_d, xts_d, scols_d, wsum_d, addc_d, c_abs):
    nc = tc.nc
    xt_pool = ctx.enter_context(tc.tile_pool(name="xtp", bufs=1))
    cst_pool = ctx.enter_context(tc.tile_pool(name="cst", bufs=1))
    g_pool = ctx.enter_context(tc.tile_pool(name="gp", bufs=2, space="PSUM"))
    acc_pool = ctx.enter_context(tc.tile_pool(name="accp", bufs=1, space="PSUM"))
    outp_pool = ctx.enter_context(tc.tile_pool(name="outpp", bufs=1, space="PSUM"))
    sq_pool = ctx.enter_context(tc.tile_pool(name="sqp", bufs=4))
    msk_pool = ctx.enter_context(tc.tile_pool(name="mskp", bufs=4))
    fin_pool = ctx.enter_context(tc.tile_pool(name="finp", bufs=2))

    # --- constants / inputs resident in SBUF ---
    xts_t = cst_pool.tile([128, MSLAB], f32, tag="xts")
    nc.sync.dma_start(out=xts_t[:], in_=xts_d[:])
    scols_t = cst_pool.tile([128, 2 * NB], bf16, tag="scols")
    nc.sync.dma_start(out=scols_t[:], in_=scols_d[:])
    wsum_t = cst_pool.tile([128, H], f32, tag="wsum")
    nc.sync.dma_start(out=wsum_t[:], in_=wsum_d[:])

    xt_tiles = []
    for j in range(N // 512):
        t = xt_pool.tile([128, 512], f32, tag=f"xt{j}")
        nc.sync.dma_start(out=t[:], in_=xt_d[:, j * 512:(j + 1) * 512])
        xt_tiles.append(t)

    groups = _groups()

    for mc in range(NPASS):
        accs = [acc_pool.tile([128, MCHUNK], f32, tag=f"acc{j}",
                               name=f"acc{j}_{mc}")
                for j in range(3)]
        rhs = xts_t[:, mc * MCHUNK:(mc + 1) * MCHUNK]
        for g, nbs in enumerate(groups):
            fd = len(nbs) * MCHUNK
            gt = g_pool.tile([128, GS * MCHUNK], f32, tag="g")
            for k, nb in enumerate(nbs):
                lhs = xt_tiles[nb // 4][:, (nb % 4) * 128:(nb % 4) * 128 + 128]
                nc.tensor.matmul(out=gt[:, k * MCHUNK:(k + 1) * MCHUNK],
                                 lhsT=lhs, rhs=rhs,
                                 start=True, stop=True)
            msk = msk_pool.tile([128, GS * MCHUNK], bf16, tag="msk")
            # ACT square-drain from PSUM, alternate compare engine
            sq = sq_pool.tile([128, GS * MCHUNK], f32, tag="sq")
            nc.scalar.activation(sq[:, :fd], gt[:, :fd], AFT.Square)
            ceng = nc.gpsimd if g % 2 == 0 else nc.vector
            ceng.tensor_scalar(msk[:, :fd], sq[:, :fd],
                               THRESH, None, AOP.is_ge)
            for k, nb in enumerate(nbs):
                # lhsT = [s_hi | s_lo] exact bf16 split of s/128
                nc.tensor.matmul(out=accs[0][0:2, :],
                                 lhsT=scols_t[:, 2 * nb:2 * nb + 2],
                                 rhs=msk[:, k * MCHUNK:(k + 1) * MCHUNK],
                                 start=(nb == 0), stop=(nb == NB - 1))

        # --- finalize this m-chunk: rank-1 expansion + fused bias/diag ---
        a = fin_pool.tile([128, MCHUNK], f32, tag="aggs0",
                          name=f"aggs0_{mc}")
        nc.scalar.copy(a[0:2, :], accs[0][0:2, :])
        for sub in range(MCHUNK // 128):
            outp = outp_pool.tile([128, H], f32, tag="outp")
            lo = sub * 128
            nc.tensor.matmul(out=outp[:],
                             lhsT=a[0:2, lo:lo + 128],
                             rhs=wsum_t[0:2, :],
                             start=True, stop=True)
            row0 = mc * MCHUNK + sub * 128
            bb = fin_pool.tile([128, H], f32, tag="bb")
            nc.sync.dma_start(out=bb[:], in_=addc_d[row0:row0 + 128, :])
            ot = fin_pool.tile([128, H], f32, tag="ot")
            nc.vector.tensor_add(ot[:], outp[:], bb[:])
            nc.sync.dma_start(out=out_d[row0:row0 + 128, :], in_=ot[:])


_CACHE = {}


def kernel(x: np.ndarray, W: np.ndarray, b: np.ndarray,
           trace: bool = False, tmpdir: str | None = None):
    x = np.asarray(x, dtype=np.float32)
    W = np.asarray(W, dtype=np.float32)
    b = np.asarray(b, dtype=np.float32)

    c_abs = _abs_threshold()

    # host-side prep (cheap, O(N*D))
    xt = np.ascontiguousarray(x.T)                       # [128, N]
    import ml_dtypes
    s = (x.astype(np.float64).sum(axis=1) / 128.0).astype(np.float32)  # [N]
    s_hi = s.astype(ml_dtypes.bfloat16)
    s_lo = (s - s_hi.astype(np.float32)).astype(ml_dtypes.bfloat16)
    # scols[p, 2b + t] = (s_hi if t==0 else s_lo)[b*128 + p]
    scols = np.empty((128, 2 * NB), dtype=ml_dtypes.bfloat16)
    scols[:, 0::2] = s_hi.reshape(NB, 128).T
    scols[:, 1::2] = s_lo.reshape(NB, 128).T
    wsum1 = W.astype(np.float64).sum(axis=1).astype(np.float32)
    wsum = np.ascontiguousarray(np.broadcast_to(wsum1[None, :], (128, H))).astype(np.float32)
    # diagonal correction: subtract s_i when fl(G_ii^2) >= 0.85
    q = np.einsum("nd,nd->n", x, x, dtype=np.float64).astype(np.float32)
    dmask = (np.float32(q * q) >= np.float32(THRESH))
    corr = np.where(dmask, s, np.float32(0.0)).astype(np.float32)  # [N]
    addc = b[None, :].astype(np.float32) - np.outer(corr, wsum[0].astype(np.float32))

    if "nc" not in _CACHE:
        _CACHE["nc"] = _build_kernel(c_abs)
    nc = _CACHE["nc"]

    in_maps = []
    for c in range(NCORES):
        sl = slice(c * MSLAB, (c + 1) * MSLAB)
        in_maps.append({
            "xt": xt,
            "xts": np.ascontiguousarray(xt[:, sl]),
            "scols": scols,
            "wsum": wsum,
            "addc": np.ascontiguousarray(addc[sl]),
        })

    res = run_bass_kernel_spmd(nc, in_maps, list(range(NCORES)),
                               trace=trace, tmpdir=tmpdir)
    out = np.concatenate([r["out"] for r in res.results], axis=0)
    if trace:
        kernel.last_results = res
    return out.astype(np.float32)



# revision 2
# speedup vs baseline: 3.9048x; 3.9048x over previous
"""GraphQLayer fused kernel for 8x trn2 NeuronCores.

Math reduction: the reference output is rank-1 per row.
  fid = (x @ x.T)^2 ; adj = (fid >= 0.85), zero diag
  agg_scalar[i] = sum_j adj[i,j] * s[j],  s[j] = sum_d x[j,d] / 128
  out[i, h] = agg_scalar[i] * wsum[h] + b[h],  wsum[h] = sum_d W[h, d].

Mask trick (single ACT pass, exact): adj_ij = 1 iff |G_ij| >= g0 where
g0 = minimal f32 y with fl(y*y) >= 0.85.  The kernel computes 2*G on the
PE (one matmul operand pre-doubled -- exact power-of-two scale), then
fin = Is_finite(2G * c) on the scalar engine with c chosen (exact rational
arithmetic) so fl(2G * c) overflows to inf iff |G| >= g0.  So fin = NOT adj
and agg_i = S - sum_j fin_ij s_j - diag_corr_i with S = sum_j s_j.

Per core (row slab of 2048): for each of 2 i-chunks (1024) x 128 j-blocks:
  2x fp32 MM (N=512) -> 2G in PSUM; ACT Is_finite -> bf16 fin in SBUF;
  2x bf16 acc-MM with lhsT = [s_hi | s_lo] columns accumulating
  accfin[2, 1024] over j.  Finalize: rank-1 MM with -wsum, add
  host-precomputed addc = b + (S - corr)*wsum, DMA out.
"""

import sys
from contextlib import ExitStack

import numpy as np

sys.path.insert(0, "/opt/trn_rl_repo")

import concourse.bass as bass  # noqa: E402
import concourse.tile as tile  # noqa: E402
from concourse import bacc, mybir  # noqa: E402
from concourse.bass_utils import run_bass_kernel_spmd  # noqa: E402

N, D, H = 16384, 128, 64
NCORES = 8
MSLAB = N // NCORES          # 2048 output rows per core
ICHUNK = 1024                # i columns per acc pass
NIC = MSLAB // ICHUNK        # 2
NB = N // 128                # 128 j-blocks
THRESH = 0.85

f32 = mybir.dt.float32
bf16 = mybir.dt.bfloat16
AOP = mybir.AluOpType
AFT = mybir.ActivationFunctionType


def _abs_threshold() -> float:
    """Minimal fp32 y such that fl(y*y) >= 0.85 (exact mask equivalence)."""
    y = np.float32(np.sqrt(np.float64(THRESH)))
    thr = np.float32(THRESH)
    while np.float32(y * y) >= thr:
        y = np.nextafter(y, np.float32(0.0))
    while np.float32(y * y) < thr:
        y = np.nextafter(y, np.float32(np.inf))
    return float(y)


def _overflow_scale(g0: float) -> float:
    """f32 c such that fl((2x)*c) overflows to inf iff |x| >= g0 (RNE)."""
    from fractions import Fraction
    T = Fraction(2**128 - 2**103)
    g2 = 2 * Fraction(np.float64(np.float32(g0)))
    pred2 = 2 * Fraction(np.float64(np.nextafter(np.float32(g0), np.float32(0.0))))
    lo = T / g2
    hi = T / pred2
    c = np.float32(float(lo))
    while Fraction(np.float64(c)) < lo:
        c = np.nextafter(c, np.float32(np.inf))
    assert Fraction(np.float64(c)) < hi, "no f32 scale in overflow window"
    assert g2 * Fraction(np.float64(c)) >= T
    assert pred2 * Fraction(np.float64(c)) < T
    return float(c)


def _build_kernel(cscale: float):
    nc = bacc.Bacc("TRN2", target_bir_lowering=False, debug=False,
                   num_devices=NCORES)
    xt_d = nc.dram_tensor("xt", [128, N], f32, kind="ExternalInput").ap()
    xts2_d = nc.dram_tensor("xts2", [128, MSLAB], f32, kind="ExternalInput").ap()
    scols_d = nc.dram_tensor("scols", [128, 2 * NB], bf16, kind="ExternalInput").ap()
    wsumn_d = nc.dram_tensor("wsumn", [128, H], f32, kind="ExternalInput").ap()
    addc_d = nc.dram_tensor("addc", [MSLAB, H], f32, kind="ExternalInput").ap()
    out_d = nc.dram_tensor("out", [MSLAB, H], f32, kind="ExternalOutput").ap()

    with tile.TileContext(nc) as tc:
        with ExitStack() as ctx:
            _emit(ctx, tc, out_d, xt_d, xts2_d, scols_d, wsumn_d, addc_d, cscale)
    nc.compile()
    return nc


def _emit(ctx, tc, out_d, xt_d, xts2_d, scols_d, wsumn_d, addc_d, cscale):
    nc = tc.nc
    xt_pool = ctx.enter_context(tc.tile_pool(name="xtp", bufs=1))
    cst_pool = ctx.enter_context(tc.tile_pool(name="cst", bufs=1))
    g_pool = ctx.enter_context(tc.tile_pool(name="gp", bufs=2, space="PSUM"))
    acc_pool = ctx.enter_context(tc.tile_pool(name="accp", bufs=1, space="PSUM"))
    outp_pool = ctx.enter_context(tc.tile_pool(name="outpp", bufs=1, space="PSUM"))
    msk_pool = ctx.enter_context(tc.tile_pool(name="mskp", bufs=4))
    fin_pool = ctx.enter_context(tc.tile_pool(name="finp", bufs=2))

    # --- constants / inputs resident in SBUF ---
    xts2_t = cst_pool.tile([128, MSLAB], f32, tag="xts2")
    nc.sync.dma_start(out=xts2_t[:], in_=xts2_d[:])
    scols_t = cst_pool.tile([128, 2 * NB], bf16, tag="scols")
    nc.sync.dma_start(out=scols_t[:], in_=scols_d[:])
    wsumn_t = cst_pool.tile([128, H], f32, tag="wsumn")
    nc.sync.dma_start(out=wsumn_t[:], in_=wsumn_d[:])

    xt_tiles = []
    for j in range(N // 2048):
        t = xt_pool.tile([128, 2048], f32, tag=f"xt{j}")
        nc.sync.dma_start(out=t[:], in_=xt_d[:, j * 2048:(j + 1) * 2048])
        xt_tiles.append(t)

    for ic in range(NIC):
        i0 = ic * ICHUNK
        acc = acc_pool.tile([2, ICHUNK], f32, tag="acc", name=f"acc_{ic}")
        for jb in range(NB):
            lhs = xt_tiles[jb // 16][:, (jb % 16) * 128:(jb % 16) * 128 + 128]
            gt = g_pool.tile([128, ICHUNK], f32, tag="g", name=f"g_{ic}_{jb}")
            for h in range(2):
                nc.tensor.matmul(
                    out=gt[:, h * 512:(h + 1) * 512],
                    lhsT=lhs,
                    rhs=xts2_t[:, i0 + h * 512:i0 + (h + 1) * 512],
                    start=True, stop=True)
            msk = msk_pool.tile([128, ICHUNK], bf16, tag="m", name=f"m_{ic}_{jb}")
            nc.scalar.activation(msk[:], gt[:], AFT.Is_finite, scale=cscale)
            for h in range(2):
                nc.tensor.matmul(
                    out=acc[:, h * 512:(h + 1) * 512],
                    lhsT=scols_t[:, 2 * jb:2 * jb + 2],
                    rhs=msk[:, h * 512:(h + 1) * 512],
                    start=(jb == 0), stop=(jb == NB - 1))

        # --- finalize this i-chunk: rank-1 expansion + fused bias/diag ---
        a = fin_pool.tile([128, ICHUNK], f32, tag="a", name=f"a_{ic}")
        nc.scalar.copy(a[0:2, :], acc[0:2, :])
        for sub in range(ICHUNK // 128):
            outp = outp_pool.tile([128, H], f32, tag="outp")
            lo = sub * 128
            nc.tensor.matmul(out=outp[:],
                             lhsT=a[0:2, lo:lo + 128],
                             rhs=wsumn_t[0:2, :],
                             start=True, stop=True)
            row0 = i0 + lo
            bb = fin_pool.tile([128, H], f32, tag="bb")
            nc.sync.dma_start(out=bb[:], in_=addc_d[row0:row0 + 128, :])
            ot = fin_pool.tile([128, H], f32, tag="ot")
            nc.vector.tensor_add(ot[:], outp[:], bb[:])
            nc.sync.dma_start(out=out_d[row0:row0 + 128, :], in_=ot[:])


_CACHE = {}


def kernel(x: np.ndarray, W: np.ndarray, b: np.ndarray,
           trace: bool = False, tmpdir: str | None = None):
    x = np.asarray(x, dtype=np.float32)
    W = np.asarray(W, dtype=np.float32)
    b = np.asarray(b, dtype=np.float32)

    cscale = _overflow_scale(_abs_threshold())

    # host-side prep (cheap, O(N*D))
    xt = np.ascontiguousarray(x.T)                       # [128, N]
    import ml_dtypes
    s = (x.astype(np.float64).sum(axis=1) / 128.0).astype(np.float32)  # [N]
    s_hi = s.astype(ml_dtypes.bfloat16)
    s_lo = (s - s_hi.astype(np.float32)).astype(ml_dtypes.bfloat16)
    # scols[p, 2b + t] = (s_hi if t==0 else s_lo)[b*128 + p]
    scols = np.empty((128, 2 * NB), dtype=ml_dtypes.bfloat16)
    scols[:, 0::2] = s_hi.reshape(NB, 128).T
    scols[:, 1::2] = s_lo.reshape(NB, 128).T
    S = np.float32(s.astype(np.float64).sum())
    wsum1 = W.astype(np.float64).sum(axis=1).astype(np.float32)  # [H]
    wsumn = np.ascontiguousarray(
        np.broadcast_to(-wsum1[None, :], (128, H))).astype(np.float32)
    # diagonal correction: subtract s_i when fl(q^2) >= 0.85 (always, in
    # practice, since q = |x_i|^2 ~ 128)
    q = np.einsum("nd,nd->n", x, x, dtype=np.float64).astype(np.float32)
    dmask = (np.float32(q * q) >= np.float32(THRESH))
    corr = np.where(dmask, s, np.float32(0.0)).astype(np.float32)  # [N]
    # out = addc - (sum_j fin s_j) * wsum ;  addc = b + (S - corr) * wsum
    addc = (b[None, :].astype(np.float32)
            + np.outer((S - corr).astype(np.float32), wsum1)).astype(np.float32)

    if "nc" not in _CACHE:
        _CACHE["nc"] = _build_kernel(cscale)
    nc = _CACHE["nc"]

    in_maps = []
    for c in range(NCORES):
        sl = slice(c * MSLAB, (c + 1) * MSLAB)
        in_maps.append({
            "xt": xt,
            "xts2": np.ascontiguousarray(2.0 * xt[:, sl]),
            "scols": scols,
            "wsumn": wsumn,
            "addc": np.ascontiguousarray(addc[sl]),
        })

    res = run_bass_kernel_spmd(nc, in_maps, list(range(NCORES)),
                               trace=trace, tmpdir=tmpdir)
    out = np.concatenate([r["out"] for r in res.results], axis=0)
    if trace:
        kernel.last_results = res
    return out.astype(np.float32)
